# revision 9
# baseline (speedup 1.0000x reference)
"""AttentionBlock kernel for Trainium2 (8 NeuronCores, batch-sharded).

Per sample b:
    q = Wq @ x + bq            [32, N]
    k = Wk @ x + bk            [32, N]
    v = Wv @ x                 [256, N]   (v-bias folded: sum((v+bv)p) = sum(vp) + bv*den)
    attn = softmax(q^T k)      [N, N] (softmax over keys)
    out = gamma * (v @ attn^T) + x

fp8 pipeline: exp(logit - c_chunk) is written directly as fp8e5m2
(unnormalized softmax weights; c_chunk is a host-computed per-chunk
shift that keeps the largest weight ~e^10.5 < e5m2 max 57344 and the
weakest query's top key above the denormal cutoff).  PV and the
denominator ones-matmul then run in fp8 DoubleRow mode (2 keys per PE
cell, 2x throughput, contraction 256/matmul).  The N^2 exp is split
across two engines: ACT does exp->e5m2 with the per-chunk shift as the
activation bias; DVE computes the same e5m2 bits with one
tensor_scalar op (Schraudolph: uint8 = rne_saturate(logit*4/ln2 +
(60 - 4c/ln2)) bitcast e5m2; fp32->u8 saturation gives exact softmax
underflow-to-zero, verified on HW).  The denominator lands replicated
on 32 partitions from the DR ones-matmul, so normalization is
reciprocal + DMA partition-replicate + one multiply per output tile;
sign(gamma) is folded into Wv, |gamma| into the denominator, and
gamma*bv into x (on the Pool engine) ahead of the epilogue add.
"""

from contextlib import ExitStack

import numpy as np

import concourse.bass as bass
import concourse.mybir as mybir
import concourse.tile as tile
from concourse import bacc
from concourse.bass_utils import run_bass_kernel_spmd

B, C, H, W = 8, 256, 64, 64
N = H * W        # 4096
D = 32           # C // 8
NCORES = 8
P = 128
F32 = mybir.dt.float32
F32R = mybir.dt.float32r
BF16 = mybir.dt.bfloat16
U8 = mybir.dt.uint8
E5 = mybir.dt.float8e5
E4 = mybir.dt.float8e4

NW = 8           # n-chunks of 512 queries
NCH = N // NW    # 512
MP = N // P      # 32 key-chunks of 128
QUAD = 4         # key-chunks per group (row-packed S^T)
NG = MP // QUAD  # 8 groups
NKP = MP // 2    # 16 key-pairs (DoubleRow)

A_S = 4.0 / float(np.log(2.0))   # schraudolph scale for e5m2 bits
MARGIN = 10.5                    # c_chunk = chunk_max_logit - MARGIN
EPS_DEN = 1e-6

Exp = mybir.ActivationFunctionType.Exp
Copy = mybir.ActivationFunctionType.Copy
MUL = mybir.AluOpType.mult
ADD = mybir.AluOpType.add
DR = mybir.MatmulPerfMode.DoubleRow

# exp-tile engine split: tile A (key-chunks 4g,4g+1) -> ACT; tile B -> DVE,
# except every EXPB_ACT_EVERYth B-tile also goes to ACT for load balance.
EXPB_ACT_EVERY = 6


def build_bass(reps=1):
    nc = bacc.Bacc("TRN2", target_bir_lowering=False, debug=False,
                   enable_asserts=False, num_devices=NCORES)

    x_d = nc.dram_tensor("x", [C, N], F32R, kind="ExternalInput").ap()
    wqT_d = nc.dram_tensor("wqT", [C, D], F32R, kind="ExternalInput").ap()
    wkT_d = nc.dram_tensor("wkT", [C, D], F32R, kind="ExternalInput").ap()
    wvT_d = nc.dram_tensor("wvT", [C, C], F32R, kind="ExternalInput").ap()
    bq_d = nc.dram_tensor("bq", [D, 1], F32, kind="ExternalInput").ap()
    bk_d = nc.dram_tensor("bk", [D, 1], F32, kind="ExternalInput").ap()
    eb_d = nc.dram_tensor("eb", [P, NW], F32, kind="ExternalInput").ap()
    sb8_d = nc.dram_tensor("sb8", [P, NW], F32, kind="ExternalInput").ap()
    gbv_d = nc.dram_tensor("gbv", [P, 2], F32, kind="ExternalInput").ap()
    igam_d = nc.dram_tensor("igam", [D, 1], F32, kind="ExternalInput").ap()
    ones8_d = nc.dram_tensor("ones8", [P, 2, D], E5, kind="ExternalInput").ap()
    out_d = nc.dram_tensor("out", [C, N], F32, kind="ExternalOutput").ap()

    with tile.TileContext(nc) as tc, ExitStack() as ctx:
        const = ctx.enter_context(tc.tile_pool(name="const", bufs=1))
        xp = ctx.enter_context(tc.tile_pool(name="xp", bufs=1))
        qk = ctx.enter_context(tc.tile_pool(name="qk", bufs=1))
        vt = ctx.enter_context(tc.tile_pool(name="vt", bufs=1))
        pt = ctx.enter_context(tc.tile_pool(name="pt", bufs=7))
        dn = ctx.enter_context(tc.tile_pool(name="dn", bufs=2))
        rp = ctx.enter_context(tc.tile_pool(name="rp", bufs=2))
        ob = ctx.enter_context(tc.tile_pool(name="ob", bufs=2))
        ps_st = ctx.enter_context(tc.tile_pool(name="ps_st", bufs=2, space="PSUM"))
        ps_out = ctx.enter_context(tc.tile_pool(name="ps_out", bufs=1, space="PSUM"))
        ps_den = ctx.enter_context(tc.tile_pool(name="ps_den", bufs=2, space="PSUM"))

        for _rep in range(reps):
            # ---- load inputs ----
            wqT_sb = const.tile([P, 2, D], F32R)
            nc.sync.dma_start(out=wqT_sb[:, 0, :], in_=wqT_d[0:P, :])
            nc.sync.dma_start(out=wqT_sb[:, 1, :], in_=wqT_d[P:C, :])
            wkT_sb = const.tile([P, 2, D], F32R)
            nc.sync.dma_start(out=wkT_sb[:, 0, :], in_=wkT_d[0:P, :])
            nc.sync.dma_start(out=wkT_sb[:, 1, :], in_=wkT_d[P:C, :])
            wvT_sb = const.tile([P, 2, C], F32R)
            nc.sync.dma_start(out=wvT_sb[:, 0, :], in_=wvT_d[0:P, :])
            nc.sync.dma_start(out=wvT_sb[:, 1, :], in_=wvT_d[P:C, :])
            bq_sb = const.tile([D, 1], F32)
            nc.sync.dma_start(out=bq_sb, in_=bq_d)
            bk_sb = const.tile([D, 1], F32)
            nc.sync.dma_start(out=bk_sb, in_=bk_d)
            eb_sb = const.tile([P, NW], F32)
            nc.sync.dma_start(out=eb_sb, in_=eb_d)
            sb8_sb = const.tile([P, NW], F32)
            nc.sync.dma_start(out=sb8_sb, in_=sb8_d)
            gbv_sb = const.tile([P, 2], F32)
            nc.sync.dma_start(out=gbv_sb, in_=gbv_d)
            igam_sb = const.tile([D, 1], F32)
            nc.sync.dma_start(out=igam_sb, in_=igam_d)
            ones8_sb = const.tile([P, 2, D], E5)
            nc.sync.dma_start(out=ones8_sb, in_=ones8_d)

            x_sb = xp.tile([P, 2, N], F32R)           # [128, c-half, 4096]
            for j in range(NW):
                sl = slice(j * NCH, (j + 1) * NCH)
                for ci in range(2):
                    nc.sync.dma_start(out=x_sb[:, ci, sl],
                                      in_=x_d[ci * P:(ci + 1) * P, sl])

            # ---- prologue ----
            q_pack = qk.tile([P, N], BF16)
            k_sb = qk.tile([D, N], BF16)
            k_pack = qk.tile([P, NG, P], BF16)
            v8_sb = vt.tile([P, NKP, 2, C], E4)       # [128, keypair, pair, chan]

            _pro = [(ps_st, "stq"), (ps_out, "outq"), (ps_den, "den")]

            def pro_ps(idx, shape, tag_pair):
                pool, tg = _pro[idx % 3]
                return pool.tile(shape, F32, name=f"pro_{tag_pair}_{idx}", tag=tg)

            for j in range(NW):
                sl = slice(j * NCH, (j + 1) * NCH)
                ps_q = pro_ps(j, [D, NCH], "q")
                for ci in range(2):
                    nc.tensor.matmul(ps_q, lhsT=wqT_sb[:, ci, :],
                                     rhs=x_sb[:, ci, sl],
                                     start=(ci == 0), stop=(ci == 1))
                nc.vector.tensor_scalar_add(out=q_pack[0:D, sl], in0=ps_q,
                                            scalar1=bq_sb)
                ps_k = pro_ps(j + 1, [D, NCH], "k")
                for ci in range(2):
                    nc.tensor.matmul(ps_k, lhsT=wkT_sb[:, ci, :],
                                     rhs=x_sb[:, ci, sl],
                                     start=(ci == 0), stop=(ci == 1))
                nc.vector.tensor_scalar_add(out=k_sb[:, sl], in0=ps_k,
                                            scalar1=bk_sb)

            # replicate q to partition groups 1..3; scatter k into k_pack
            for j in range(1, 4):
                nc.sync.dma_start(out=q_pack[D * j:D * (j + 1), :],
                                  in_=q_pack[0:D, :])
            k_view = k_sb.rearrange("p (g j c) -> p g j c", g=NG, j=QUAD, c=P)
            for j in range(4):
                nc.sync.dma_start(out=k_pack[D * j:D * (j + 1), :, :],
                                  in_=k_view[:, :, j, :])

            for kp in range(NKP):
                ps_v = pro_ps(kp, [P, 2, C], "v")
                for mi in range(2):
                    m = kp * 2 + mi
                    msl = slice(m * P, (m + 1) * P)
                    for ci in range(2):
                        nc.tensor.matmul(ps_v[:, mi, :], lhsT=x_sb[:, ci, msl],
                                         rhs=wvT_sb[:, ci, :],
                                         start=(ci == 0), stop=(ci == 1))
                nc.scalar.activation(out=v8_sb[:, kp, :, :], in_=ps_v,
                                     func=Copy)

            # ---- main attention loop ----
            expb_ctr = 0
            for n in range(NW):
                nsl = slice(n * NCH, (n + 1) * NCH)
                out_ps = ps_out.tile([P, 2, NCH], F32, tag="outq")
                den_ps = ps_den.tile([D, NCH], F32, tag="den")
                pend = {}
                for g in range(NG + 1):
                    if g < NG:
                        st_a = ps_st.tile([P, 2, NCH], F32, tag="stq")
                        st_b = ps_st.tile([P, 2, NCH], F32, tag="stq")
                        for j in range(QUAD):
                            dst = st_a if j < 2 else st_b
                            nc.tensor.matmul(dst[:, j % 2, :],
                                             lhsT=k_pack[D * j:D * (j + 1), g, :],
                                             rhs=q_pack[D * j:D * (j + 1), nsl],
                                             start=True, stop=True,
                                             tile_position=(D * j, 0))
                        p_a = pt.tile([P, 2, NCH], E5)
                        nc.scalar.activation(out=p_a, in_=st_a, func=Exp,
                                             bias=eb_sb[:, n:n + 1])
                        p_b = pt.tile([P, 2, NCH], E5)
                        expb_ctr += 1
                        if expb_ctr % EXPB_ACT_EVERY == 0:
                            nc.scalar.activation(out=p_b, in_=st_b, func=Exp,
                                                 bias=eb_sb[:, n:n + 1])
                        else:
                            nc.vector.tensor_scalar(out=p_b.bitcast(U8),
                                                    in0=st_b, scalar1=A_S,
                                                    scalar2=sb8_sb[:, n:n + 1],
                                                    op0=MUL, op1=ADD)
                        pend[g] = (p_a, p_b)
                    if g > 0:
                        gg = g - 1
                        p_a, p_b = pend.pop(gg)
                        first = (gg == 0)
                        last = (gg == NG - 1)
                        for i, pp in enumerate((p_a, p_b)):
                            kp = 2 * gg + i
                            st = first and i == 0
                            sp = last and i == 1
                            nc.tensor.matmul(den_ps, lhsT=ones8_sb, rhs=pp,
                                             perf_mode=DR, start=st, stop=sp)
                            for h in range(2):
                                nc.tensor.matmul(out_ps[:, h, :],
                                                 lhsT=v8_sb[:, kp, :,
                                                            h * P:(h + 1) * P],
                                                 rhs=pp, perf_mode=DR,
                                                 start=st, stop=sp)
                # rd = |gamma| / den, replicated to 128 partitions
                den_sb = dn.tile([D, NCH], F32, name=f"den_sb_{n}")
                nc.vector.tensor_scalar(out=den_sb, in0=den_ps,
                                        scalar1=igam_sb, scalar2=EPS_DEN,
                                        op0=MUL, op1=ADD)
                rd_sb = rp.tile([P, NCH], F32, name=f"rd_{n}")
                nc.vector.reciprocal_approx_fast(out=rd_sb[0:D, :], in_=den_sb)
                nc.sync.dma_start(out=rd_sb[D:2 * D, :], in_=rd_sb[0:D, :])
                nc.sync.dma_start(out=rd_sb[2 * D:P, :], in_=rd_sb[0:2 * D, :])
                out_sb = ob.tile([P, 2, NCH], F32)
                for h in range(2):
                    nc.vector.tensor_tensor(out=out_sb[:, h, :],
                                            in0=out_ps[:, h, :], in1=rd_sb,
                                            op=MUL)
                    nc.gpsimd.tensor_scalar(out=out_sb[:, h, :],
                                            in0=out_sb[:, h, :],
                                            scalar1=gbv_sb[:, h:h + 1],
                                            scalar2=None, op0=ADD)
                    nc.gpsimd.tensor_tensor(out=out_sb[:, h, :],
                                            in0=out_sb[:, h, :],
                                            in1=x_sb[:, h, nsl].bitcast(F32),
                                            op=ADD)
                    nc.sync.dma_start(out=out_d[h * P:(h + 1) * P, nsl],
                                      in_=out_sb[:, h, :])
    nc.compile()
    return nc


_NC_CACHE = None


def _get_nc():
    global _NC_CACHE
    if _NC_CACHE is None:
        _NC_CACHE = build_bass()
    return _NC_CACHE


def _in_maps(inputs):
    import ml_dtypes
    x = np.ascontiguousarray(np.asarray(inputs["x"], dtype=np.float32))
    Wq = np.asarray(inputs["Wq"], np.float32)
    Wk = np.asarray(inputs["Wk"], np.float32)
    wqT = np.ascontiguousarray(Wq.T)
    wkT = np.ascontiguousarray(Wk.T)
    bq = np.asarray(inputs["bq"], np.float32).reshape(D, 1).copy()
    bk = np.asarray(inputs["bk"], np.float32).reshape(D, 1).copy()
    bv = np.asarray(inputs["bv"], np.float32)
    gamma = float(np.asarray(inputs["gamma"], np.float32).reshape(()))
    sg = 1.0 if gamma >= 0 else -1.0
    wvT = np.ascontiguousarray(np.asarray(inputs["Wv"], np.float32).T * sg)
    igam = np.full((D, 1), 1.0 / max(abs(gamma), 1e-12), np.float32)
    gbv = np.ascontiguousarray((gamma * bv).reshape(2, P).T)  # [P, 2]
    ones8 = np.ones((P, 2, D), np.float32).astype(ml_dtypes.float8_e5m2)

    # per-sample, per-chunk logit shift from bf16 q/k (as the device computes)
    xf = x.reshape(B, C, N)
    qh = (np.einsum('dc,bcn->bdn', Wq, xf) + bq.reshape(1, D, 1))
    kh = (np.einsum('dc,bcn->bdn', Wk, xf) + bk.reshape(1, D, 1))
    qh = qh.astype(ml_dtypes.bfloat16).astype(np.float32)
    kh = kh.astype(ml_dtypes.bfloat16).astype(np.float32)

    maps = []
    for b in range(NCORES):
        S = qh[b].T @ kh[b]                     # [queries, keys]
        cmax = S.reshape(NW, NCH, N).max(axis=(1, 2))  # per-chunk max logit
        c = (cmax - MARGIN).astype(np.float32)
        eb = np.broadcast_to(-c[None, :], (P, NW)).copy()
        sb8 = np.broadcast_to((60.0 - A_S * c)[None, :], (P, NW)).astype(np.float32).copy()
        maps.append({
            "x": np.ascontiguousarray(xf[b]),
            "wqT": wqT, "wkT": wkT, "wvT": wvT,
            "bq": bq, "bk": bk, "eb": eb, "sb8": sb8,
            "gbv": gbv, "igam": igam, "ones8": ones8,
        })
    return maps


def _run(inputs, **kw):
    nc = _get_nc()
    res = run_bass_kernel_spmd(nc, _in_maps(inputs), core_ids=list(range(NCORES)),
                               **kw)
    outs = [res.results[b]["out"].reshape(C, H, W) for b in range(NCORES)]
    return np.stack(outs, axis=0).astype(np.float32), res


def kernel(**inputs) -> np.ndarray:
    out, _ = _run(inputs)
    return out
